# revision 13
# baseline (speedup 1.0000x reference)
"""Trainium2 Bass kernel for nn_AttentionBlock_88983132438589.

Math (value-level, all stop_gradient tricks removed):
  q = x @ Wq.T + bq ; k = x @ Wk.T + bk ; v = x @ Wv.T + bv
  scores = q @ k.T per head (NO 1/sqrt(d) scaling)
  probs  = softmax(scores)           (gamma_AH cancels in value)
  ctx    = probs @ v
  h      = ctx @ (Wo + gLN*relu(Wo)).T + (bo + gLN*relu(bo)) + x
  out    = (h - mean(h)) / sqrt(var(h) + 1e-12)
  returns (out, probs)

Sharding: data-parallel over batch, 1 batch element per NeuronCore (8 cores).

On-device layout is "transposed": Q^T,K^T [o, s] so the scores matmul
contracts over head_dim on partitions, softmax denominators come from an
ones-column appended to V in the ctx matmul (they land in ctx row 64), and
ctx^T directly feeds the output projection as the stationary operand.
probs are written to HBM as [h, k, q] (transposed); host transposes back.
"""

import sys
import numpy as np

sys.path.insert(0, "/opt/trn_rl_repo")

B, S, H = 8, 1024, 1024
NH, HD = 16, 64
P = 128
EPS = 1e-12

_CACHE = {}


def _build(probs_bf16=True):
    import concourse.bass as bass  # noqa
    import concourse.mybir as mybir
    import concourse.tile as tile
    from concourse import bacc

    f32 = mybir.dt.float32
    f32r = mybir.dt.float32r
    bf16 = mybir.dt.bfloat16
    AF = mybir.ActivationFunctionType
    OP = mybir.AluOpType
    pdt = bf16 if probs_bf16 else f32

    nc = bacc.Bacc("TRN2", target_bir_lowering=False, debug=False, num_devices=8)

    xt = nc.dram_tensor("xt", [H, S], f32, kind="ExternalInput")
    xn = nc.dram_tensor("xn", [S, H], f32, kind="ExternalInput")
    wqt = nc.dram_tensor("wqt", [H, H], f32, kind="ExternalInput")
    wkt = nc.dram_tensor("wkt", [H, H], f32, kind="ExternalInput")
    wvt = nc.dram_tensor("wvt", [H, H], f32, kind="ExternalInput")
    wot = nc.dram_tensor("wot", [H, H], f32, kind="ExternalInput")
    bq = nc.dram_tensor("bq", [H], f32, kind="ExternalInput")
    bk = nc.dram_tensor("bk", [H], f32, kind="ExternalInput")
    bv = nc.dram_tensor("bv", [H], f32, kind="ExternalInput")
    bo = nc.dram_tensor("bo", [H], f32, kind="ExternalInput")
    g = nc.dram_tensor("g", [1, 1], f32, kind="ExternalInput")

    out = nc.dram_tensor("out", [S, H], f32, kind="ExternalOutput")
    probs_t = nc.dram_tensor("probs_t", [NH, S, S], pdt, kind="ExternalOutput")

    with tile.TileContext(nc) as tc:
        constp = tc.alloc_tile_pool(name="constp", bufs=1)
        rows = tc.alloc_tile_pool(name="rows", bufs=1)

        bq_sb = constp.tile([P, 8], f32)
        nc.sync.dma_start(bq_sb[:], bq.ap().rearrange("(t p) -> p t", p=P))
        bk_sb = constp.tile([P, 8], f32)
        nc.sync.dma_start(bk_sb[:], bk.ap().rearrange("(t p) -> p t", p=P))
        g_sb = constp.tile([1, 1], f32)
        nc.sync.dma_start(g_sb[:], g.ap())
        g128 = constp.tile([P, 1], f32)
        nc.gpsimd.partition_broadcast(g128[:], g_sb[:])
        eps_sb = constp.tile([P, 1], f32)
        nc.vector.memset(eps_sb[:], EPS)

        bv_row = rows.tile([1, H], f32)
        nc.sync.dma_start(bv_row[:], bv.ap()[None, :])
        bv_b = constp.tile([P, H], f32)
        nc.gpsimd.partition_broadcast(bv_b[:], bv_row[:])
        bo_row = rows.tile([1, H], f32)
        nc.sync.dma_start(bo_row[:], bo.ap()[None, :])
        bo_relu = rows.tile([1, H], f32)
        nc.scalar.activation(bo_relu[:], bo_row[:], AF.Relu)
        bo_p_row = rows.tile([1, H], f32)
        nc.vector.scalar_tensor_tensor(
            bo_p_row[:], bo_relu[:], g_sb[:], bo_row[:], OP.mult, OP.add)
        bo_b = constp.tile([P, H], f32)
        nc.gpsimd.partition_broadcast(bo_b[:], bo_p_row[:])
        rows.release()

        qkvp = tc.alloc_tile_pool(name="qkvp", bufs=1)
        qt_sb = qkvp.tile([P, 8, S], f32r)
        kt_sb = qkvp.tile([P, 8, S], f32r)
        v_sb = qkvp.tile([P, 8, NH * 65], bf16)

        ph1 = tc.alloc_tile_pool(name="ph1", bufs=1)
        wpool = tc.alloc_tile_pool(name="wpool", bufs=2)
        ph1ps = tc.alloc_tile_pool(name="ph1ps", bufs=6, space="PSUM")

        xt_sb = ph1.tile([P, 8, S], f32r)
        xt_r = xt.ap().rearrange("(t p) s -> p t s", p=P)
        wv_sb = wpool.tile([P, 8, H], f32r, tag="w", name="wv_sb")
        wv_r = wvt.ap().rearrange("(t p) o -> p t o", p=P)
        for it in range(8):
            nc.gpsimd.dma_start(xt_sb[:, it], xt_r[:, it])
            nc.gpsimd.dma_start(wv_sb[:, it], wv_r[:, it])
        for st in range(8):
            ones_ap = v_sb[:, st].rearrange("p (h d) -> p h d", d=65)[:, :, 64:65]
            nc.vector.memset(ones_ap, 1.0)

        for st in range(8):
            for oc in range(2):
                psv = ph1ps.tile([P, 512], f32, tag="pj", name="psv")
                for it in range(8):
                    nc.tensor.matmul(
                        psv[:], xt_sb[:, it, st * P:(st + 1) * P],
                        wv_sb[:, it, oc * 512:(oc + 1) * 512],
                        start=(it == 0), stop=(it == 7))
                v_out = v_sb[:, st].rearrange(
                    "p (h d) -> p h d", d=65)[:, oc * 8:(oc + 1) * 8, 0:64]
                nc.vector.tensor_tensor(
                    v_out,
                    psv[:].rearrange("p (h d) -> p h d", d=64),
                    bv_b[:, oc * 512:(oc + 1) * 512].rearrange(
                        "p (h d) -> p h d", d=64),
                    OP.add)

        wq_sb = wpool.tile([P, 8, H], f32r, tag="w", name="wq_sb")
        wq_r = wqt.ap().rearrange("(t p) o -> p t o", p=P)
        wk_sb = wpool.tile([P, 8, H], f32r, tag="w", name="wk_sb")
        wk_r = wkt.ap().rearrange("(t p) o -> p t o", p=P)
        for it in range(8):
            nc.gpsimd.dma_start(wq_sb[:, it], wq_r[:, it])
            nc.gpsimd.dma_start(wk_sb[:, it], wk_r[:, it])
        for ot in range(8):
            for w_sb, slab, bias in ((wq_sb, qt_sb, bq_sb), (wk_sb, kt_sb, bk_sb)):
                pst = [ph1ps.tile([P, 512], f32, tag="pj", name="pst")
                       for _ in range(2)]
                for it in range(8):
                    lhsT = w_sb[:, it, ot * P:(ot + 1) * P]
                    for sc in range(2):
                        nc.tensor.matmul(
                            pst[sc][:], lhsT,
                            xt_sb[:, it, sc * 512:(sc + 1) * 512],
                            start=(it == 0), stop=(it == 7))
                for sc in range(2):
                    nc.vector.tensor_scalar_add(
                        slab[:, ot, sc * 512:(sc + 1) * 512],
                        pst[sc][:], bias[:, ot:ot + 1])

        ph1ps.release()
        wpool.release()
        ph1.release()

        persist = tc.alloc_tile_pool(name="persist", bufs=1, side="right")
        ctxt_sb = persist.tile([P, 8, H], f32r)
        xn_sb = persist.tile([P, 8, H], f32)

        ph2 = tc.alloc_tile_pool(name="ph2", bufs=2)
        epool = tc.alloc_tile_pool(name="epool", bufs=14)
        stps = tc.alloc_tile_pool(name="stps", bufs=2, space="PSUM")
        ctxps = tc.alloc_tile_pool(name="ctxps", bufs=4, space="PSUM")

        nc.sync.dma_start(xn_sb[:], xn.ap().rearrange("(t p) o -> p t o", p=P))

        for qc in range(2):
            qsl = slice(qc * 512, (qc + 1) * 512)
            for hp in range(8):
                heads = [(2 * hp, 0), (2 * hp + 1, 64)]
                ctx_ps = [ctxps.tile([65, 512], f32, tag="ctx", name="ctx_ps")
                          for _ in range(2)]
                e_tiles = [[None] * 4, [None] * 4]
                for pp in range(4):
                    st_ps = [stps.tile([P, 1024], f32, tag="st", name="st_ps")
                             for _ in range(2)]
                    for gg in range(2):
                        kt = 2 * pp + gg
                        ksl = slice(kt * P, (kt + 1) * P)
                        for hi, (h, po) in enumerate(heads):
                            nc.tensor.matmul(
                                st_ps[hi][:, gg * 512:(gg + 1) * 512],
                                kt_sb[po:po + 64, hp, ksl],
                                qt_sb[po:po + 64, hp, qsl],
                                start=True, stop=True)
                    for hi, (h, po) in enumerate(heads):
                        e_t = epool.tile([P, 2, 512], bf16, tag="E", name="e_t")
                        nc.scalar.activation(
                            e_t[:],
                            st_ps[hi][:].rearrange("p (g q) -> p g q", g=2),
                            AF.Exp)
                        e_tiles[hi][pp] = e_t
                    for gg in range(2):
                        kt = 2 * pp + gg
                        for hi, (h, po) in enumerate(heads):
                            nc.tensor.matmul(
                                ctx_ps[hi][:],
                                v_sb[:, kt, h * 65:(h + 1) * 65],
                                e_tiles[hi][pp][:, gg],
                                start=(kt == 0), stop=(kt == 7))
                for hi, (h, po) in enumerate(heads):
                    lnrow = ph2.tile([1, 512], f32, tag="lnrow", name="lnrow")
                    nc.scalar.activation(lnrow[:], ctx_ps[hi][64:65, :], AF.Ln)
                    rrow = ph2.tile([1, 512], bf16, tag="rrow", name="rrow")
                    nc.scalar.activation(rrow[:], lnrow[:], AF.Exp, scale=-1.0)
                    rb = ph2.tile([P, 512], bf16, tag="rb", name="rb")
                    nc.gpsimd.partition_broadcast(rb[:], rrow[:])
                    nc.vector.tensor_tensor(
                        ctxt_sb[po:po + 64, hp, qsl],
                        ctx_ps[hi][0:64, :], rb[0:64, :], OP.mult)
                    for pp in range(4):
                        e_t = e_tiles[hi][pp]
                        for gg in range(2):
                            nc.vector.tensor_tensor(
                                e_t[:, gg], e_t[:, gg], rb[:], OP.mult)
                        dst = probs_t.ap()[
                            h, pp * 256:(pp + 1) * 256, qsl
                        ].rearrange("(g r) q -> r g q", g=2)
                        nc.sync.dma_start(dst, e_t[:])

        ctxps.release()
        stps.release()
        epool.release()
        ph2.release()
        qkvp.release()

        ph3 = tc.alloc_tile_pool(name="ph3", bufs=1)
        ph3tmp = tc.alloc_tile_pool(name="ph3tmp", bufs=2)
        ph3ps = tc.alloc_tile_pool(name="ph3ps", bufs=4, space="PSUM")

        wot_sb = ph3.tile([P, 8, H], f32)
        wot_r = wot.ap().rearrange("(t p) o -> p t o", p=P)
        wotp_sb = ph3.tile([P, 8, H], f32r)
        for it in range(8):
            nc.sync.dma_start(wot_sb[:, it], wot_r[:, it])
            tmp = ph3tmp.tile([P, H], f32, tag="relu", name="tmp")
            nc.scalar.activation(tmp[:], wot_sb[:, it], AF.Relu)
            nc.vector.scalar_tensor_tensor(
                wotp_sb[:, it], tmp[:], g128[:, 0:1], wot_sb[:, it],
                OP.mult, OP.add)
        for st in range(8):
            nc.vector.tensor_tensor(xn_sb[:, st], xn_sb[:, st], bo_b[:], OP.add)

        out_r = out.ap().rearrange("(t p) o -> t p o", p=P)
        for st in range(8):
            pso = [ph3ps.tile([P, 512], f32, tag="o", name="pso")
                   for _ in range(2)]
            for it in range(8):
                lhsT = ctxt_sb[:, it, st * P:(st + 1) * P]
                for oc in range(2):
                    nc.tensor.matmul(
                        pso[oc][:], lhsT,
                        wotp_sb[:, it, oc * 512:(oc + 1) * 512],
                        start=(it == 0), stop=(it == 7))
            h_sb = ph3tmp.tile([P, H], f32, tag="h", name="h_sb")
            for oc in range(2):
                nc.vector.tensor_tensor(
                    h_sb[:, oc * 512:(oc + 1) * 512], pso[oc][:],
                    xn_sb[:, st, oc * 512:(oc + 1) * 512], OP.add)
            stats = ph3tmp.tile([P, 2, 6], f32, tag="stats", name="stats")
            for g2 in range(2):
                nc.vector.bn_stats(stats[:, g2], h_sb[:, g2 * 512:(g2 + 1) * 512])
            mv = ph3tmp.tile([P, 2], f32, tag="mv", name="mv")
            nc.vector.bn_aggr(mv[:], stats[:])
            lnv = ph3tmp.tile([P, 1], f32, tag="lnv", name="lnv")
            nc.scalar.activation(lnv[:], mv[:, 1:2], AF.Ln, bias=eps_sb[:, 0:1])
            y = ph3tmp.tile([P, 1], f32, tag="y", name="y")
            nc.scalar.activation(y[:], lnv[:], AF.Exp, scale=-0.5)
            t1 = ph3tmp.tile([P, 1], f32, tag="t1", name="t1")
            nc.vector.tensor_tensor(t1[:], y[:], y[:], OP.mult)
            nc.vector.tensor_tensor(t1[:], t1[:], mv[:, 1:2], OP.mult)
            nc.vector.tensor_scalar(t1[:], t1[:], -0.5, 1.5, OP.mult, OP.add)
            nc.vector.tensor_tensor(y[:], y[:], t1[:], OP.mult)
            negm = ph3tmp.tile([P, 1], f32, tag="negm", name="negm")
            nc.vector.tensor_scalar_mul(negm[:], mv[:, 0:1], -1.0)
            o_sb = ph3tmp.tile([P, H], f32, tag="o_sb", name="o_sb")
            nc.vector.tensor_scalar(
                o_sb[:], h_sb[:], negm[:, 0:1], y[:, 0:1], OP.add, OP.mult)
            nc.sync.dma_start(out_r[st], o_sb[:])

        ph3ps.release()
        ph3tmp.release()
        ph3.release()
        persist.release()
        constp.release()

    nc.compile()
    return nc


def _get_nc(probs_bf16=True):
    key = ("nc", probs_bf16)
    if key not in _CACHE:
        _CACHE[key] = _build(probs_bf16)
    return _CACHE[key]


def run(inputs, trace=False, probs_bf16=True):
    from concourse.bass_utils import run_bass_kernel_spmd

    nc = _get_nc(probs_bf16)
    X = np.ascontiguousarray(np.asarray(inputs["hidden_states"], np.float32))
    wqt = np.ascontiguousarray(np.asarray(inputs["Wq"], np.float32).T)
    wkt = np.ascontiguousarray(np.asarray(inputs["Wk"], np.float32).T)
    wvt = np.ascontiguousarray(np.asarray(inputs["Wv"], np.float32).T)
    wot = np.ascontiguousarray(np.asarray(inputs["Wo"], np.float32).T)
    bq = np.ascontiguousarray(np.asarray(inputs["bq"], np.float32))
    bk = np.ascontiguousarray(np.asarray(inputs["bk"], np.float32))
    bv = np.ascontiguousarray(np.asarray(inputs["bv"], np.float32))
    bo = np.ascontiguousarray(np.asarray(inputs["bo"], np.float32))
    gl = np.array([[np.float32(inputs["gamma_LN"])]], np.float32)

    in_maps = []
    for b in range(B):
        in_maps.append({
            "xt": np.ascontiguousarray(X[b].T),
            "xn": X[b],
            "wqt": wqt, "wkt": wkt, "wvt": wvt, "wot": wot,
            "bq": bq, "bk": bk, "bv": bv, "bo": bo,
            "g": gl,
        })
    res = run_bass_kernel_spmd(nc, in_maps, core_ids=list(range(B)),
                               trace=trace)
    out = np.stack([res.results[b]["out"] for b in range(B)])
    probs = np.stack([
        np.asarray(res.results[b]["probs_t"]).astype(np.float32).transpose(0, 2, 1)
        for b in range(B)
    ])
    return (out, probs), res


def kernel(**inputs):
    (out, probs), _ = run(inputs, trace=False)
    return out, probs


# revision 16
# speedup vs baseline: 1.0514x; 1.0514x over previous
"""Trainium2 Bass kernel for nn_AttentionBlock_88983132438589.

Math (value-level, all stop_gradient tricks removed):
  q = x @ Wq.T + bq ; k = x @ Wk.T + bk ; v = x @ Wv.T + bv
  scores = q @ k.T per head (NO 1/sqrt(d) scaling)
  probs  = softmax(scores)           (gamma_AH cancels in value)
  ctx    = probs @ v
  h      = ctx @ (Wo + gLN*relu(Wo)).T + (bo + gLN*relu(bo)) + x
  out    = (h - mean(h)) / sqrt(var(h) + 1e-12)
  returns (out, probs)

Sharding: data-parallel over batch, 1 batch element per NeuronCore (8 cores).

On-device layout is "transposed": Q^T,K^T [o, s] so the scores matmul
contracts over head_dim on partitions, softmax denominators come from an
ones-column appended to V in the ctx matmul (they land in ctx row 64), and
ctx^T directly feeds the output projection as the stationary operand.
probs are written to HBM as [h, k, q] (transposed); host transposes back.
"""

import sys
import numpy as np

sys.path.insert(0, "/opt/trn_rl_repo")

B, S, H = 8, 1024, 1024
NH, HD = 16, 64
P = 128
EPS = 1e-12

_CACHE = {}


def _build(probs_bf16=True):
    import concourse.bass as bass  # noqa
    import concourse.mybir as mybir
    import concourse.tile as tile
    from concourse import bacc

    f32 = mybir.dt.float32
    f32r = mybir.dt.float32r
    bf16 = mybir.dt.bfloat16
    AF = mybir.ActivationFunctionType
    OP = mybir.AluOpType
    pdt = bf16 if probs_bf16 else f32

    nc = bacc.Bacc("TRN2", target_bir_lowering=False, debug=False, num_devices=8)

    xt = nc.dram_tensor("xt", [H, S], f32, kind="ExternalInput")
    xn = nc.dram_tensor("xn", [S, H], f32, kind="ExternalInput")
    wqt = nc.dram_tensor("wqt", [H, H], f32, kind="ExternalInput")
    wkt = nc.dram_tensor("wkt", [H, H], f32, kind="ExternalInput")
    wvt = nc.dram_tensor("wvt", [H, H], f32, kind="ExternalInput")
    wot = nc.dram_tensor("wot", [H, H], f32, kind="ExternalInput")
    bq = nc.dram_tensor("bq", [H], f32, kind="ExternalInput")
    bk = nc.dram_tensor("bk", [H], f32, kind="ExternalInput")
    bv = nc.dram_tensor("bv", [H], f32, kind="ExternalInput")
    bo = nc.dram_tensor("bo", [H], f32, kind="ExternalInput")
    g = nc.dram_tensor("g", [1, 1], f32, kind="ExternalInput")

    out = nc.dram_tensor("out", [S, H], f32, kind="ExternalOutput")
    probs_t = nc.dram_tensor("probs_t", [NH, S, S], pdt, kind="ExternalOutput")

    with tile.TileContext(nc) as tc:
        constp = tc.alloc_tile_pool(name="constp", bufs=1)
        rows = tc.alloc_tile_pool(name="rows", bufs=1)

        bq_sb = constp.tile([P, 8], f32)
        nc.sync.dma_start(bq_sb[:], bq.ap().rearrange("(t p) -> p t", p=P))
        bk_sb = constp.tile([P, 8], f32)
        nc.sync.dma_start(bk_sb[:], bk.ap().rearrange("(t p) -> p t", p=P))
        g_sb = constp.tile([1, 1], f32)
        nc.sync.dma_start(g_sb[:], g.ap())
        g128 = constp.tile([P, 1], f32)
        nc.gpsimd.partition_broadcast(g128[:], g_sb[:])
        eps_sb = constp.tile([P, 1], f32)
        nc.vector.memset(eps_sb[:], EPS)

        bv_row = rows.tile([1, H], f32)
        nc.sync.dma_start(bv_row[:], bv.ap()[None, :])
        bv_b = constp.tile([P, H], f32)
        nc.gpsimd.partition_broadcast(bv_b[:], bv_row[:])
        bo_row = rows.tile([1, H], f32)
        nc.sync.dma_start(bo_row[:], bo.ap()[None, :])
        bo_relu = rows.tile([1, H], f32)
        nc.scalar.activation(bo_relu[:], bo_row[:], AF.Relu)
        bo_p_row = rows.tile([1, H], f32)
        nc.vector.scalar_tensor_tensor(
            bo_p_row[:], bo_relu[:], g_sb[:], bo_row[:], OP.mult, OP.add)
        bo_b = constp.tile([P, H], f32)
        nc.gpsimd.partition_broadcast(bo_b[:], bo_p_row[:])
        rows.release()

        qkvp = tc.alloc_tile_pool(name="qkvp", bufs=1)
        qt_sb = qkvp.tile([P, 8, S], f32r)
        kt_sb = qkvp.tile([P, 8, S], f32r)
        v_sb = qkvp.tile([P, 8, NH * 65], bf16)

        stps = tc.alloc_tile_pool(name="stps", bufs=2, space="PSUM")
        ctxps = tc.alloc_tile_pool(name="ctxps", bufs=4, space="PSUM")

        ph1 = tc.alloc_tile_pool(name="ph1", bufs=1)
        wpool = tc.alloc_tile_pool(name="wpool", bufs=2)

        xt_sb = ph1.tile([P, 8, S], f32r)
        xt_r = xt.ap().rearrange("(t p) s -> p t s", p=P)
        wv_sb = wpool.tile([P, 8, H], f32r, tag="w", name="wv_sb")
        wv_r = wvt.ap().rearrange("(t p) o -> p t o", p=P)
        for it in range(8):
            nc.gpsimd.dma_start(xt_sb[:, it], xt_r[:, it])
            nc.gpsimd.dma_start(wv_sb[:, it], wv_r[:, it])
        for st in range(8):
            ones_ap = v_sb[:, st].rearrange("p (h d) -> p h d", d=65)[:, :, 64:65]
            nc.vector.memset(ones_ap, 1.0)

        for st in range(8):
            for oc in range(2):
                psv = ctxps.tile([P, 512], f32, tag="pj", name="psv")
                for it in range(8):
                    nc.tensor.matmul(
                        psv[:], xt_sb[:, it, st * P:(st + 1) * P],
                        wv_sb[:, it, oc * 512:(oc + 1) * 512],
                        start=(it == 0), stop=(it == 7))
                v_out = v_sb[:, st].rearrange(
                    "p (h d) -> p h d", d=65)[:, oc * 8:(oc + 1) * 8, 0:64]
                nc.vector.tensor_tensor(
                    v_out,
                    psv[:].rearrange("p (h d) -> p h d", d=64),
                    bv_b[:, oc * 512:(oc + 1) * 512].rearrange(
                        "p (h d) -> p h d", d=64),
                    OP.add)

        wq_sb = wpool.tile([P, 8, H], f32r, tag="w", name="wq_sb")
        wq_r = wqt.ap().rearrange("(t p) o -> p t o", p=P)
        wk_sb = wpool.tile([P, 8, H], f32r, tag="w", name="wk_sb")
        wk_r = wkt.ap().rearrange("(t p) o -> p t o", p=P)
        for it in range(8):
            nc.gpsimd.dma_start(wq_sb[:, it], wq_r[:, it])
            nc.gpsimd.dma_start(wk_sb[:, it], wk_r[:, it])
        for ot in range(8):
            for w_sb, slab, bias in ((wq_sb, qt_sb, bq_sb), (wk_sb, kt_sb, bk_sb)):
                pst = [ctxps.tile([P, 512], f32, tag="pj", name="pst")
                       for _ in range(2)]
                for it in range(8):
                    lhsT = w_sb[:, it, ot * P:(ot + 1) * P]
                    for sc in range(2):
                        nc.tensor.matmul(
                            pst[sc][:], lhsT,
                            xt_sb[:, it, sc * 512:(sc + 1) * 512],
                            start=(it == 0), stop=(it == 7))
                for sc in range(2):
                    nc.vector.tensor_scalar_add(
                        slab[:, ot, sc * 512:(sc + 1) * 512],
                        pst[sc][:], bias[:, ot:ot + 1])

        wpool.release()
        ph1.release()

        persist = tc.alloc_tile_pool(name="persist", bufs=1, side="right")
        ctxt_sb = persist.tile([P, 8, H], f32r)
        xn_sb = persist.tile([P, 8, H], f32)

        ph2 = tc.alloc_tile_pool(name="ph2", bufs=3)
        epool = tc.alloc_tile_pool(name="epool", bufs=14)

        nc.sync.dma_start(xn_sb[:], xn.ap().rearrange("(t p) o -> p t o", p=P))

        for qc in range(2):
            qsl = slice(qc * 512, (qc + 1) * 512)
            for hp in range(8):
                heads = [(2 * hp, 0), (2 * hp + 1, 64)]
                ctx_ps = [ctxps.tile([65, 512], f32, tag="pj", name="ctx_ps")
                          for _ in range(2)]
                e_tiles = [[None] * 4, [None] * 4]
                for pp in range(4):
                    st_ps = [stps.tile([P, 1024], f32, tag="st", name="st_ps")
                             for _ in range(2)]
                    for gg in range(2):
                        kt = 2 * pp + gg
                        ksl = slice(kt * P, (kt + 1) * P)
                        for hi, (h, po) in enumerate(heads):
                            nc.tensor.matmul(
                                st_ps[hi][:, gg * 512:(gg + 1) * 512],
                                kt_sb[po:po + 64, hp, ksl],
                                qt_sb[po:po + 64, hp, qsl],
                                start=True, stop=True)
                    for hi, (h, po) in enumerate(heads):
                        e_t = epool.tile([P, 2, 512], bf16, tag="E", name="e_t")
                        nc.scalar.activation(
                            e_t[:],
                            st_ps[hi][:].rearrange("p (g q) -> p g q", g=2),
                            AF.Exp)
                        e_tiles[hi][pp] = e_t
                    for gg in range(2):
                        kt = 2 * pp + gg
                        for hi, (h, po) in enumerate(heads):
                            nc.tensor.matmul(
                                ctx_ps[hi][:],
                                v_sb[:, kt, h * 65:(h + 1) * 65],
                                e_tiles[hi][pp][:, gg],
                                start=(kt == 0), stop=(kt == 7))
                # batch Ln then Exp across the head pair: 2 ACT table
                # switches per hp-iteration instead of 4
                lnrows = [ph2.tile([1, 512], f32, tag="lnrow", name="lnrow")
                          for _ in range(2)]
                rrows = [ph2.tile([1, 512], bf16, tag="rrow", name="rrow")
                         for _ in range(2)]
                with tc.tile_critical():
                    for hi in range(2):
                        nc.scalar.activation(lnrows[hi][:],
                                             ctx_ps[hi][64:65, :], AF.Ln)
                    for hi in range(2):
                        nc.scalar.activation(rrows[hi][:], lnrows[hi][:],
                                             AF.Exp, scale=-1.0)
                for hi, (h, po) in enumerate(heads):
                    rb = ph2.tile([P, 512], bf16, tag="rb", name="rb")
                    nc.gpsimd.partition_broadcast(rb[:], rrows[hi][:])
                    nc.vector.tensor_tensor(
                        ctxt_sb[po:po + 64, hp, qsl],
                        ctx_ps[hi][0:64, :], rb[0:64, :], OP.mult)
                    for pp in range(4):
                        e_t = e_tiles[hi][pp]
                        for gg in range(2):
                            nc.vector.tensor_tensor(
                                e_t[:, gg], e_t[:, gg], rb[:], OP.mult)
                        dst = probs_t.ap()[
                            h, pp * 256:(pp + 1) * 256, qsl
                        ].rearrange("(g r) q -> r g q", g=2)
                        nc.sync.dma_start(dst, e_t[:])

        epool.release()
        ph2.release()
        qkvp.release()

        ph3 = tc.alloc_tile_pool(name="ph3", bufs=1)
        ph3tmp = tc.alloc_tile_pool(name="ph3tmp", bufs=2)

        wot_sb = ph3.tile([P, 8, H], f32)
        wot_r = wot.ap().rearrange("(t p) o -> p t o", p=P)
        wotp_sb = ph3.tile([P, 8, H], f32r)
        for it in range(8):
            nc.sync.dma_start(wot_sb[:, it], wot_r[:, it])
            tmp = ph3tmp.tile([P, H], f32, tag="relu", name="tmp")
            nc.scalar.activation(tmp[:], wot_sb[:, it], AF.Relu)
            nc.vector.scalar_tensor_tensor(
                wotp_sb[:, it], tmp[:], g128[:, 0:1], wot_sb[:, it],
                OP.mult, OP.add)
        for st in range(8):
            nc.vector.tensor_tensor(xn_sb[:, st], xn_sb[:, st], bo_b[:], OP.add)

        out_r = out.ap().rearrange("(t p) o -> t p o", p=P)
        for st in range(8):
            pso = [ctxps.tile([P, 512], f32, tag="pj", name="pso")
                   for _ in range(2)]
            for it in range(8):
                lhsT = ctxt_sb[:, it, st * P:(st + 1) * P]
                for oc in range(2):
                    nc.tensor.matmul(
                        pso[oc][:], lhsT,
                        wotp_sb[:, it, oc * 512:(oc + 1) * 512],
                        start=(it == 0), stop=(it == 7))
            h_sb = ph3tmp.tile([P, H], f32, tag="h", name="h_sb")
            for oc in range(2):
                nc.vector.tensor_tensor(
                    h_sb[:, oc * 512:(oc + 1) * 512], pso[oc][:],
                    xn_sb[:, st, oc * 512:(oc + 1) * 512], OP.add)
            stats = ph3tmp.tile([P, 2, 6], f32, tag="stats", name="stats")
            for g2 in range(2):
                nc.vector.bn_stats(stats[:, g2], h_sb[:, g2 * 512:(g2 + 1) * 512])
            mv = ph3tmp.tile([P, 2], f32, tag="mv", name="mv")
            nc.vector.bn_aggr(mv[:], stats[:])
            lnv = ph3tmp.tile([P, 1], f32, tag="lnv", name="lnv")
            nc.scalar.activation(lnv[:], mv[:, 1:2], AF.Ln, bias=eps_sb[:, 0:1])
            y = ph3tmp.tile([P, 1], f32, tag="y", name="y")
            nc.scalar.activation(y[:], lnv[:], AF.Exp, scale=-0.5)
            t1 = ph3tmp.tile([P, 1], f32, tag="t1", name="t1")
            nc.vector.tensor_tensor(t1[:], y[:], y[:], OP.mult)
            nc.vector.tensor_tensor(t1[:], t1[:], mv[:, 1:2], OP.mult)
            nc.vector.tensor_scalar(t1[:], t1[:], -0.5, 1.5, OP.mult, OP.add)
            nc.vector.tensor_tensor(y[:], y[:], t1[:], OP.mult)
            negm = ph3tmp.tile([P, 1], f32, tag="negm", name="negm")
            nc.vector.tensor_scalar_mul(negm[:], mv[:, 0:1], -1.0)
            o_sb = ph3tmp.tile([P, H], f32, tag="o_sb", name="o_sb")
            nc.vector.tensor_scalar(
                o_sb[:], h_sb[:], negm[:, 0:1], y[:, 0:1], OP.add, OP.mult)
            nc.sync.dma_start(out_r[st], o_sb[:])

        ph3tmp.release()
        ph3.release()
        ctxps.release()
        stps.release()
        persist.release()
        constp.release()

    nc.compile()
    return nc


def _get_nc(probs_bf16=True):
    key = ("nc", probs_bf16)
    if key not in _CACHE:
        _CACHE[key] = _build(probs_bf16)
    return _CACHE[key]


def run(inputs, trace=False, probs_bf16=True):
    from concourse.bass_utils import run_bass_kernel_spmd

    nc = _get_nc(probs_bf16)
    X = np.ascontiguousarray(np.asarray(inputs["hidden_states"], np.float32))
    wqt = np.ascontiguousarray(np.asarray(inputs["Wq"], np.float32).T)
    wkt = np.ascontiguousarray(np.asarray(inputs["Wk"], np.float32).T)
    wvt = np.ascontiguousarray(np.asarray(inputs["Wv"], np.float32).T)
    wot = np.ascontiguousarray(np.asarray(inputs["Wo"], np.float32).T)
    bq = np.ascontiguousarray(np.asarray(inputs["bq"], np.float32))
    bk = np.ascontiguousarray(np.asarray(inputs["bk"], np.float32))
    bv = np.ascontiguousarray(np.asarray(inputs["bv"], np.float32))
    bo = np.ascontiguousarray(np.asarray(inputs["bo"], np.float32))
    gl = np.array([[np.float32(inputs["gamma_LN"])]], np.float32)

    in_maps = []
    for b in range(B):
        in_maps.append({
            "xt": np.ascontiguousarray(X[b].T),
            "xn": X[b],
            "wqt": wqt, "wkt": wkt, "wvt": wvt, "wot": wot,
            "bq": bq, "bk": bk, "bv": bv, "bo": bo,
            "g": gl,
        })
    res = run_bass_kernel_spmd(nc, in_maps, core_ids=list(range(B)),
                               trace=trace)
    out = np.stack([res.results[b]["out"] for b in range(B)])
    probs = np.stack([
        np.asarray(res.results[b]["probs_t"]).astype(np.float32).transpose(0, 2, 1)
        for b in range(B)
    ])
    return (out, probs), res


def kernel(**inputs):
    (out, probs), _ = run(inputs, trace=False)
    return out, probs


# revision 17
# speedup vs baseline: 1.4100x; 1.3411x over previous
"""Trainium2 Bass kernel for nn_AttentionBlock_88983132438589.

Math (value-level, all stop_gradient tricks removed):
  q = x @ Wq.T + bq ; k = x @ Wk.T + bk ; v = x @ Wv.T + bv
  scores = q @ k.T per head (NO 1/sqrt(d) scaling)
  probs  = softmax(scores)           (gamma_AH cancels in value)
  ctx    = probs @ v
  h      = ctx @ (Wo + gLN*relu(Wo)).T + (bo + gLN*relu(bo)) + x
  out    = (h - mean(h)) / sqrt(var(h) + 1e-12)
  returns (out, probs)

Sharding: data-parallel over batch, 1 batch element per NeuronCore (8 cores).

On-device layout is "transposed": Q^T,K^T [o, s] so the scores matmul
contracts over head_dim on partitions, softmax denominators come from an
ones-column appended to V in the ctx matmul (they land in ctx row 64), and
ctx^T directly feeds the output projection as the stationary operand.
probs are written to HBM as [h, k, q] (transposed); host transposes back.
"""

import sys
import numpy as np

sys.path.insert(0, "/opt/trn_rl_repo")

B, S, H = 8, 1024, 1024
NH, HD = 16, 64
P = 128
EPS = 1e-12

_CACHE = {}


def _build(probs_bf16=True):
    import concourse.bass as bass  # noqa
    import concourse.mybir as mybir
    import concourse.tile as tile
    from concourse import bacc

    f32 = mybir.dt.float32
    f32r = mybir.dt.float32r
    bf16 = mybir.dt.bfloat16
    AF = mybir.ActivationFunctionType
    OP = mybir.AluOpType
    pdt = bf16 if probs_bf16 else f32

    nc = bacc.Bacc("TRN2", target_bir_lowering=False, debug=False, num_devices=8)

    xt = nc.dram_tensor("xt", [H, S], f32, kind="ExternalInput")
    xn = nc.dram_tensor("xn", [S, H], f32, kind="ExternalInput")
    wqt = nc.dram_tensor("wqt", [H, H], f32, kind="ExternalInput")
    wkt = nc.dram_tensor("wkt", [H, H], f32, kind="ExternalInput")
    wvt = nc.dram_tensor("wvt", [H, H], f32, kind="ExternalInput")
    wot = nc.dram_tensor("wot", [H, H], f32, kind="ExternalInput")
    bq = nc.dram_tensor("bq", [H], f32, kind="ExternalInput")
    bk = nc.dram_tensor("bk", [H], f32, kind="ExternalInput")
    bv = nc.dram_tensor("bv", [H], f32, kind="ExternalInput")
    bo = nc.dram_tensor("bo", [H], f32, kind="ExternalInput")
    g = nc.dram_tensor("g", [1, 1], f32, kind="ExternalInput")

    out = nc.dram_tensor("out", [S, H], f32, kind="ExternalOutput")
    probs_t = nc.dram_tensor("probs_t", [NH, S, S], pdt, kind="ExternalOutput")

    with tile.TileContext(nc) as tc:
        constp = tc.alloc_tile_pool(name="constp", bufs=1)
        rows = tc.alloc_tile_pool(name="rows", bufs=1)

        bq_sb = constp.tile([P, 8], f32)
        nc.sync.dma_start(bq_sb[:], bq.ap().rearrange("(t p) -> p t", p=P))
        bk_sb = constp.tile([P, 8], f32)
        nc.sync.dma_start(bk_sb[:], bk.ap().rearrange("(t p) -> p t", p=P))
        g_sb = constp.tile([1, 1], f32)
        nc.sync.dma_start(g_sb[:], g.ap())
        g128 = constp.tile([P, 1], f32)
        nc.gpsimd.partition_broadcast(g128[:], g_sb[:])
        eps_sb = constp.tile([P, 1], f32)
        nc.vector.memset(eps_sb[:], EPS)

        bv_row = rows.tile([1, H], f32)
        nc.sync.dma_start(bv_row[:], bv.ap()[None, :])
        bv_b = constp.tile([P, H], f32)
        nc.gpsimd.partition_broadcast(bv_b[:], bv_row[:])
        bo_row = rows.tile([1, H], f32)
        nc.sync.dma_start(bo_row[:], bo.ap()[None, :])
        bo_relu = rows.tile([1, H], f32)
        nc.scalar.activation(bo_relu[:], bo_row[:], AF.Relu)
        bo_p_row = rows.tile([1, H], f32)
        nc.vector.scalar_tensor_tensor(
            bo_p_row[:], bo_relu[:], g_sb[:], bo_row[:], OP.mult, OP.add)
        bo_b = constp.tile([P, H], f32)
        nc.gpsimd.partition_broadcast(bo_b[:], bo_p_row[:])
        rows.release()

        qkvp = tc.alloc_tile_pool(name="qkvp", bufs=1)
        qt_sb = qkvp.tile([P, 8, S], f32r)
        kt_sb = qkvp.tile([P, 8, S], f32r)
        v_sb = qkvp.tile([P, 8, NH * 65], bf16)

        stps = tc.alloc_tile_pool(name="stps", bufs=2, space="PSUM")
        ctxps = tc.alloc_tile_pool(name="ctxps", bufs=4, space="PSUM")

        ph1 = tc.alloc_tile_pool(name="ph1", bufs=1)
        wpool = tc.alloc_tile_pool(name="wpool", bufs=2)

        xt_sb = ph1.tile([P, 8, S], f32r)
        xt_r = xt.ap().rearrange("(t p) s -> p t s", p=P)
        wv_sb = wpool.tile([P, 8, H], f32r, tag="w", name="wv_sb")
        wv_r = wvt.ap().rearrange("(t p) o -> p t o", p=P)
        for it in range(8):
            nc.gpsimd.dma_start(xt_sb[:, it], xt_r[:, it])
            nc.gpsimd.dma_start(wv_sb[:, it], wv_r[:, it])
        for st in range(8):
            ones_ap = v_sb[:, st].rearrange("p (h d) -> p h d", d=65)[:, :, 64:65]
            nc.vector.memset(ones_ap, 1.0)

        for st in range(8):
            for oc in range(2):
                psv = ctxps.tile([P, 512], f32, tag="pj", name="psv")
                for it in range(8):
                    nc.tensor.matmul(
                        psv[:], xt_sb[:, it, st * P:(st + 1) * P],
                        wv_sb[:, it, oc * 512:(oc + 1) * 512],
                        start=(it == 0), stop=(it == 7))
                v_out = v_sb[:, st].rearrange(
                    "p (h d) -> p h d", d=65)[:, oc * 8:(oc + 1) * 8, 0:64]
                nc.vector.tensor_tensor(
                    v_out,
                    psv[:].rearrange("p (h d) -> p h d", d=64),
                    bv_b[:, oc * 512:(oc + 1) * 512].rearrange(
                        "p (h d) -> p h d", d=64),
                    OP.add)

        wq_sb = wpool.tile([P, 8, H], f32r, tag="w", name="wq_sb")
        wq_r = wqt.ap().rearrange("(t p) o -> p t o", p=P)
        wk_sb = wpool.tile([P, 8, H], f32r, tag="w", name="wk_sb")
        wk_r = wkt.ap().rearrange("(t p) o -> p t o", p=P)
        for it in range(8):
            nc.gpsimd.dma_start(wq_sb[:, it], wq_r[:, it])
            nc.gpsimd.dma_start(wk_sb[:, it], wk_r[:, it])
        for ot in range(8):
            for w_sb, slab, bias in ((wq_sb, qt_sb, bq_sb), (wk_sb, kt_sb, bk_sb)):
                pst = [ctxps.tile([P, 512], f32, tag="pj", name="pst")
                       for _ in range(2)]
                for it in range(8):
                    lhsT = w_sb[:, it, ot * P:(ot + 1) * P]
                    for sc in range(2):
                        nc.tensor.matmul(
                            pst[sc][:], lhsT,
                            xt_sb[:, it, sc * 512:(sc + 1) * 512],
                            start=(it == 0), stop=(it == 7))
                for sc in range(2):
                    nc.vector.tensor_scalar_add(
                        slab[:, ot, sc * 512:(sc + 1) * 512],
                        pst[sc][:], bias[:, ot:ot + 1])

        wpool.release()
        ph1.release()

        persist = tc.alloc_tile_pool(name="persist", bufs=1, side="right")
        ctxt_sb = persist.tile([P, 8, H], f32r)
        xn_sb = persist.tile([P, 8, H], f32)

        ph2 = tc.alloc_tile_pool(name="ph2", bufs=3)
        epool = tc.alloc_tile_pool(name="epool", bufs=14)

        nc.sync.dma_start(xn_sb[:], xn.ap().rearrange("(t p) o -> p t o", p=P))

        for qc in range(2):
            qsl = slice(qc * 512, (qc + 1) * 512)
            for hp in range(8):
                heads = [(2 * hp, 0), (2 * hp + 1, 64)]
                ctx_ps = [ctxps.tile([65, 512], f32, tag="pj", name="ctx_ps")
                          for _ in range(2)]
                e_tiles = [[None] * 4, [None] * 4]
                for pp in range(4):
                    st_ps = [stps.tile([P, 1024], f32, tag="st", name="st_ps")
                             for _ in range(2)]
                    for gg in range(2):
                        kt = 2 * pp + gg
                        ksl = slice(kt * P, (kt + 1) * P)
                        for hi, (h, po) in enumerate(heads):
                            nc.tensor.matmul(
                                st_ps[hi][:, gg * 512:(gg + 1) * 512],
                                kt_sb[po:po + 64, hp, ksl],
                                qt_sb[po:po + 64, hp, qsl],
                                start=True, stop=True)
                    for hi, (h, po) in enumerate(heads):
                        e_t = epool.tile([P, 2, 512], bf16, tag="E", name="e_t")
                        nc.scalar.activation(
                            e_t[:],
                            st_ps[hi][:].rearrange("p (g q) -> p g q", g=2),
                            AF.Exp)
                        e_tiles[hi][pp] = e_t
                    for gg in range(2):
                        kt = 2 * pp + gg
                        for hi, (h, po) in enumerate(heads):
                            nc.tensor.matmul(
                                ctx_ps[hi][:],
                                v_sb[:, kt, h * 65:(h + 1) * 65],
                                e_tiles[hi][pp][:, gg],
                                start=(kt == 0), stop=(kt == 7))
                # batch Ln then Exp across the head pair: 2 ACT table
                # switches per hp-iteration instead of 4
                # 1/sum on DVE (reciprocal_approx_fast, ~51ULP ample for
                # the bf16 probs path); ACT Copy is set-neutral so no table
                # reloads interleave with the big softmax Exps
                rrows = []
                for hi in range(2):
                    srow = ph2.tile([1, 512], f32, tag="srow", name="srow")
                    nc.scalar.activation(srow[:], ctx_ps[hi][64:65, :],
                                         AF.Copy)
                    rrow = ph2.tile([1, 512], f32, tag="rrow", name="rrow")
                    nc.vector.reciprocal_approx_fast(rrow[:], srow[:])
                    rbf = ph2.tile([1, 512], bf16, tag="rbf", name="rbf")
                    nc.vector.tensor_copy(rbf[:], rrow[:])
                    rrows.append(rbf)
                for hi, (h, po) in enumerate(heads):
                    rb = ph2.tile([P, 512], bf16, tag="rb", name="rb")
                    nc.gpsimd.partition_broadcast(rb[:], rrows[hi][:])
                    nc.vector.tensor_tensor(
                        ctxt_sb[po:po + 64, hp, qsl],
                        ctx_ps[hi][0:64, :], rb[0:64, :], OP.mult)
                    for pp in range(4):
                        e_t = e_tiles[hi][pp]
                        for gg in range(2):
                            nc.vector.tensor_tensor(
                                e_t[:, gg], e_t[:, gg], rb[:], OP.mult)
                        dst = probs_t.ap()[
                            h, pp * 256:(pp + 1) * 256, qsl
                        ].rearrange("(g r) q -> r g q", g=2)
                        nc.sync.dma_start(dst, e_t[:])

        epool.release()
        ph2.release()
        qkvp.release()

        ph3 = tc.alloc_tile_pool(name="ph3", bufs=1)
        ph3tmp = tc.alloc_tile_pool(name="ph3tmp", bufs=2)

        wot_sb = ph3.tile([P, 8, H], f32)
        wot_r = wot.ap().rearrange("(t p) o -> p t o", p=P)
        wotp_sb = ph3.tile([P, 8, H], f32r)
        for it in range(8):
            nc.sync.dma_start(wot_sb[:, it], wot_r[:, it])
            tmp = ph3tmp.tile([P, H], f32, tag="relu", name="tmp")
            nc.scalar.activation(tmp[:], wot_sb[:, it], AF.Relu)
            nc.vector.scalar_tensor_tensor(
                wotp_sb[:, it], tmp[:], g128[:, 0:1], wot_sb[:, it],
                OP.mult, OP.add)
        for st in range(8):
            nc.vector.tensor_tensor(xn_sb[:, st], xn_sb[:, st], bo_b[:], OP.add)

        out_r = out.ap().rearrange("(t p) o -> t p o", p=P)
        for st in range(8):
            pso = [ctxps.tile([P, 512], f32, tag="pj", name="pso")
                   for _ in range(2)]
            for it in range(8):
                lhsT = ctxt_sb[:, it, st * P:(st + 1) * P]
                for oc in range(2):
                    nc.tensor.matmul(
                        pso[oc][:], lhsT,
                        wotp_sb[:, it, oc * 512:(oc + 1) * 512],
                        start=(it == 0), stop=(it == 7))
            h_sb = ph3tmp.tile([P, H], f32, tag="h", name="h_sb")
            for oc in range(2):
                nc.vector.tensor_tensor(
                    h_sb[:, oc * 512:(oc + 1) * 512], pso[oc][:],
                    xn_sb[:, st, oc * 512:(oc + 1) * 512], OP.add)
            stats = ph3tmp.tile([P, 2, 6], f32, tag="stats", name="stats")
            for g2 in range(2):
                nc.vector.bn_stats(stats[:, g2], h_sb[:, g2 * 512:(g2 + 1) * 512])
            mv = ph3tmp.tile([P, 2], f32, tag="mv", name="mv")
            nc.vector.bn_aggr(mv[:], stats[:])
            lnv = ph3tmp.tile([P, 1], f32, tag="lnv", name="lnv")
            nc.scalar.activation(lnv[:], mv[:, 1:2], AF.Ln, bias=eps_sb[:, 0:1])
            y = ph3tmp.tile([P, 1], f32, tag="y", name="y")
            nc.scalar.activation(y[:], lnv[:], AF.Exp, scale=-0.5)
            t1 = ph3tmp.tile([P, 1], f32, tag="t1", name="t1")
            nc.vector.tensor_tensor(t1[:], y[:], y[:], OP.mult)
            nc.vector.tensor_tensor(t1[:], t1[:], mv[:, 1:2], OP.mult)
            nc.vector.tensor_scalar(t1[:], t1[:], -0.5, 1.5, OP.mult, OP.add)
            nc.vector.tensor_tensor(y[:], y[:], t1[:], OP.mult)
            negm = ph3tmp.tile([P, 1], f32, tag="negm", name="negm")
            nc.vector.tensor_scalar_mul(negm[:], mv[:, 0:1], -1.0)
            o_sb = ph3tmp.tile([P, H], f32, tag="o_sb", name="o_sb")
            nc.vector.tensor_scalar(
                o_sb[:], h_sb[:], negm[:, 0:1], y[:, 0:1], OP.add, OP.mult)
            nc.sync.dma_start(out_r[st], o_sb[:])

        ph3tmp.release()
        ph3.release()
        ctxps.release()
        stps.release()
        persist.release()
        constp.release()

    nc.compile()
    return nc


def _get_nc(probs_bf16=True):
    key = ("nc", probs_bf16)
    if key not in _CACHE:
        _CACHE[key] = _build(probs_bf16)
    return _CACHE[key]


def run(inputs, trace=False, probs_bf16=True):
    from concourse.bass_utils import run_bass_kernel_spmd

    nc = _get_nc(probs_bf16)
    X = np.ascontiguousarray(np.asarray(inputs["hidden_states"], np.float32))
    wqt = np.ascontiguousarray(np.asarray(inputs["Wq"], np.float32).T)
    wkt = np.ascontiguousarray(np.asarray(inputs["Wk"], np.float32).T)
    wvt = np.ascontiguousarray(np.asarray(inputs["Wv"], np.float32).T)
    wot = np.ascontiguousarray(np.asarray(inputs["Wo"], np.float32).T)
    bq = np.ascontiguousarray(np.asarray(inputs["bq"], np.float32))
    bk = np.ascontiguousarray(np.asarray(inputs["bk"], np.float32))
    bv = np.ascontiguousarray(np.asarray(inputs["bv"], np.float32))
    bo = np.ascontiguousarray(np.asarray(inputs["bo"], np.float32))
    gl = np.array([[np.float32(inputs["gamma_LN"])]], np.float32)

    in_maps = []
    for b in range(B):
        in_maps.append({
            "xt": np.ascontiguousarray(X[b].T),
            "xn": X[b],
            "wqt": wqt, "wkt": wkt, "wvt": wvt, "wot": wot,
            "bq": bq, "bk": bk, "bv": bv, "bo": bo,
            "g": gl,
        })
    res = run_bass_kernel_spmd(nc, in_maps, core_ids=list(range(B)),
                               trace=trace)
    out = np.stack([res.results[b]["out"] for b in range(B)])
    probs = np.stack([
        np.asarray(res.results[b]["probs_t"]).astype(np.float32).transpose(0, 2, 1)
        for b in range(B)
    ])
    return (out, probs), res


def kernel(**inputs):
    (out, probs), _ = run(inputs, trace=False)
    return out, probs


# revision 20
# speedup vs baseline: 1.4500x; 1.0284x over previous
"""Trainium2 Bass kernel for nn_AttentionBlock_88983132438589.

Math (value-level, all stop_gradient tricks removed):
  q = x @ Wq.T + bq ; k = x @ Wk.T + bk ; v = x @ Wv.T + bv
  scores = q @ k.T per head (NO 1/sqrt(d) scaling)
  probs  = softmax(scores)           (gamma_AH cancels in value)
  ctx    = probs @ v
  h      = ctx @ (Wo + gLN*relu(Wo)).T + (bo + gLN*relu(bo)) + x
  out    = (h - mean(h)) / sqrt(var(h) + 1e-12)
  returns (out, probs)

Sharding: data-parallel over batch, 1 batch element per NeuronCore (8 cores).

On-device layout is "transposed": Q^T,K^T [o, s] so the scores matmul
contracts over head_dim on partitions, softmax denominators come from an
ones-column appended to V in the ctx matmul (they land in ctx row 64), and
ctx^T directly feeds the output projection as the stationary operand.
probs are written to HBM as [h, k, q] (transposed); host transposes back.
"""

import sys
import numpy as np

sys.path.insert(0, "/opt/trn_rl_repo")

B, S, H = 8, 1024, 1024
NH, HD = 16, 64
P = 128
EPS = 1e-12

_CACHE = {}


def _build(probs_bf16=True):
    import concourse.bass as bass  # noqa
    import concourse.mybir as mybir
    import concourse.tile as tile
    from concourse import bacc

    f32 = mybir.dt.float32
    f32r = mybir.dt.float32r
    bf16 = mybir.dt.bfloat16
    AF = mybir.ActivationFunctionType
    OP = mybir.AluOpType
    pdt = bf16 if probs_bf16 else f32

    nc = bacc.Bacc("TRN2", target_bir_lowering=False, debug=False, num_devices=8)

    xt = nc.dram_tensor("xt", [H, S], f32, kind="ExternalInput")
    xn = nc.dram_tensor("xn", [S, H], f32, kind="ExternalInput")
    wqt = nc.dram_tensor("wqt", [H, H], f32, kind="ExternalInput")
    wkt = nc.dram_tensor("wkt", [H, H], f32, kind="ExternalInput")
    wvt = nc.dram_tensor("wvt", [H, H], f32, kind="ExternalInput")
    wot = nc.dram_tensor("wot", [H, H], f32, kind="ExternalInput")
    bq = nc.dram_tensor("bq", [H], f32, kind="ExternalInput")
    bk = nc.dram_tensor("bk", [H], f32, kind="ExternalInput")
    bv = nc.dram_tensor("bv", [H], f32, kind="ExternalInput")
    bo = nc.dram_tensor("bo", [H], f32, kind="ExternalInput")
    g = nc.dram_tensor("g", [1, 1], f32, kind="ExternalInput")

    out = nc.dram_tensor("out", [S, H], f32, kind="ExternalOutput")
    probs_t = nc.dram_tensor("probs_t", [NH, S, S], pdt, kind="ExternalOutput")

    with tile.TileContext(nc) as tc:
        constp = tc.alloc_tile_pool(name="constp", bufs=1)
        rows = tc.alloc_tile_pool(name="rows", bufs=1)

        bq_sb = constp.tile([P, 8], f32)
        nc.sync.dma_start(bq_sb[:], bq.ap().rearrange("(t p) -> p t", p=P))
        bk_sb = constp.tile([P, 8], f32)
        nc.sync.dma_start(bk_sb[:], bk.ap().rearrange("(t p) -> p t", p=P))
        g_sb = constp.tile([1, 1], f32)
        nc.sync.dma_start(g_sb[:], g.ap())
        g128 = constp.tile([P, 1], f32)
        nc.gpsimd.partition_broadcast(g128[:], g_sb[:])
        eps_sb = constp.tile([P, 1], f32)
        nc.vector.memset(eps_sb[:], EPS)

        bv_row = rows.tile([1, H], f32)
        nc.sync.dma_start(bv_row[:], bv.ap()[None, :])
        bv_b = constp.tile([P, H], f32)
        nc.gpsimd.partition_broadcast(bv_b[:], bv_row[:])
        bo_row = rows.tile([1, H], f32)
        nc.sync.dma_start(bo_row[:], bo.ap()[None, :])
        bo_relu = rows.tile([1, H], f32)
        nc.scalar.activation(bo_relu[:], bo_row[:], AF.Relu)
        bo_p_row = rows.tile([1, H], f32)
        nc.vector.scalar_tensor_tensor(
            bo_p_row[:], bo_relu[:], g_sb[:], bo_row[:], OP.mult, OP.add)
        bo_b = constp.tile([P, H], f32)
        nc.gpsimd.partition_broadcast(bo_b[:], bo_p_row[:])
        rows.release()

        qkvp = tc.alloc_tile_pool(name="qkvp", bufs=1)
        qt_sb = qkvp.tile([P, 8, S], f32r)
        kt_sb = qkvp.tile([P, 8, S], f32r)
        v_sb = qkvp.tile([P, 8, NH * 65], bf16)

        stps = tc.alloc_tile_pool(name="stps", bufs=2, space="PSUM")
        ctxps = tc.alloc_tile_pool(name="ctxps", bufs=4, space="PSUM")

        ph1 = tc.alloc_tile_pool(name="ph1", bufs=1)
        wpool = tc.alloc_tile_pool(name="wpool", bufs=2)

        xt_sb = ph1.tile([P, 8, S], f32r)
        xt_r = xt.ap().rearrange("(t p) s -> p t s", p=P)
        wv_sb = wpool.tile([P, 8, H], f32r, tag="w", name="wv_sb")
        wv_r = wvt.ap().rearrange("(t p) o -> p t o", p=P)
        for it in range(8):
            nc.gpsimd.dma_start(xt_sb[:, it], xt_r[:, it])
            nc.gpsimd.dma_start(wv_sb[:, it], wv_r[:, it])
        for st in range(8):
            ones_ap = v_sb[:, st].rearrange("p (h d) -> p h d", d=65)[:, :, 64:65]
            nc.vector.memset(ones_ap, 1.0)

        for st in range(8):
            for oc in range(2):
                psv = ctxps.tile([P, 512], f32, tag="pj", name="psv")
                for it in range(8):
                    nc.tensor.matmul(
                        psv[:], xt_sb[:, it, st * P:(st + 1) * P],
                        wv_sb[:, it, oc * 512:(oc + 1) * 512],
                        start=(it == 0), stop=(it == 7))
                v_out = v_sb[:, st].rearrange(
                    "p (h d) -> p h d", d=65)[:, oc * 8:(oc + 1) * 8, 0:64]
                nc.vector.tensor_tensor(
                    v_out,
                    psv[:].rearrange("p (h d) -> p h d", d=64),
                    bv_b[:, oc * 512:(oc + 1) * 512].rearrange(
                        "p (h d) -> p h d", d=64),
                    OP.add)

        wq_sb = wpool.tile([P, 8, H], f32r, tag="w", name="wq_sb")
        wq_r = wqt.ap().rearrange("(t p) o -> p t o", p=P)
        wk_sb = wpool.tile([P, 8, H], f32r, tag="w", name="wk_sb")
        wk_r = wkt.ap().rearrange("(t p) o -> p t o", p=P)
        for it in range(8):
            nc.gpsimd.dma_start(wq_sb[:, it], wq_r[:, it])
            nc.gpsimd.dma_start(wk_sb[:, it], wk_r[:, it])
        for ot in range(8):
            for w_sb, slab, bias in ((wq_sb, qt_sb, bq_sb), (wk_sb, kt_sb, bk_sb)):
                pst = [ctxps.tile([P, 512], f32, tag="pj", name="pst")
                       for _ in range(2)]
                for it in range(8):
                    lhsT = w_sb[:, it, ot * P:(ot + 1) * P]
                    for sc in range(2):
                        nc.tensor.matmul(
                            pst[sc][:], lhsT,
                            xt_sb[:, it, sc * 512:(sc + 1) * 512],
                            start=(it == 0), stop=(it == 7))
                for sc in range(2):
                    nc.vector.tensor_scalar_add(
                        slab[:, ot, sc * 512:(sc + 1) * 512],
                        pst[sc][:], bias[:, ot:ot + 1])

        wpool.release()
        ph1.release()

        persist = tc.alloc_tile_pool(name="persist", bufs=1, side="right")
        ctxt_sb = persist.tile([P, 8, H], f32r)

        wotpool = tc.alloc_tile_pool(name="wotpool", bufs=1, side="right")
        wot_sb = wotpool.tile([P, 8, H], f32)
        wot_r = wot.ap().rearrange("(t p) o -> p t o", p=P)
        for it in range(8):
            nc.sync.dma_start(wot_sb[:, it], wot_r[:, it])

        ph2 = tc.alloc_tile_pool(name="ph2", bufs=3)
        epool = tc.alloc_tile_pool(name="epool", bufs=18)

        for qc in range(2):
            qsl = slice(qc * 512, (qc + 1) * 512)
            for hp in range(8):
                heads = [(2 * hp, 0), (2 * hp + 1, 64)]
                ctx_ps = [ctxps.tile([65, 512], f32, tag="pj", name="ctx_ps")
                          for _ in range(2)]
                e_tiles = [[None] * 4, [None] * 4]
                for pp in range(4):
                    st_ps = [stps.tile([P, 1024], f32, tag="st", name="st_ps")
                             for _ in range(2)]
                    for gg in range(2):
                        kt = 2 * pp + gg
                        ksl = slice(kt * P, (kt + 1) * P)
                        for hi, (h, po) in enumerate(heads):
                            nc.tensor.matmul(
                                st_ps[hi][:, gg * 512:(gg + 1) * 512],
                                kt_sb[po:po + 64, hp, ksl],
                                qt_sb[po:po + 64, hp, qsl],
                                start=True, stop=True)
                    for hi, (h, po) in enumerate(heads):
                        e_t = epool.tile([P, 2, 512], bf16, tag="E", name="e_t")
                        nc.scalar.activation(
                            e_t[:],
                            st_ps[hi][:].rearrange("p (g q) -> p g q", g=2),
                            AF.Exp)
                        e_tiles[hi][pp] = e_t
                    for gg in range(2):
                        kt = 2 * pp + gg
                        for hi, (h, po) in enumerate(heads):
                            nc.tensor.matmul(
                                ctx_ps[hi][:],
                                v_sb[:, kt, h * 65:(h + 1) * 65],
                                e_tiles[hi][pp][:, gg],
                                start=(kt == 0), stop=(kt == 7))
                # batch Ln then Exp across the head pair: 2 ACT table
                # switches per hp-iteration instead of 4
                # 1/sum on DVE (reciprocal_approx_fast, ~51ULP ample for
                # the bf16 probs path); ACT Copy is set-neutral so no table
                # reloads interleave with the big softmax Exps
                rrows = []
                for hi in range(2):
                    srow = ph2.tile([1, 512], f32, tag="srow", name="srow")
                    nc.scalar.activation(srow[:], ctx_ps[hi][64:65, :],
                                         AF.Copy)
                    rrow = ph2.tile([1, 512], f32, tag="rrow", name="rrow")
                    nc.vector.reciprocal_approx_fast(rrow[:], srow[:])
                    rbf = ph2.tile([1, 512], bf16, tag="rbf", name="rbf")
                    nc.vector.tensor_copy(rbf[:], rrow[:])
                    rrows.append(rbf)
                for hi, (h, po) in enumerate(heads):
                    rb = ph2.tile([P, 512], bf16, tag="rb", name="rb")
                    nc.gpsimd.partition_broadcast(rb[:], rrows[hi][:])
                    nc.vector.tensor_tensor(
                        ctxt_sb[po:po + 64, hp, qsl],
                        ctx_ps[hi][0:64, :], rb[0:64, :], OP.mult)
                    for pp in range(4):
                        e_t = e_tiles[hi][pp]
                        for gg in range(2):
                            nc.vector.tensor_tensor(
                                e_t[:, gg], e_t[:, gg], rb[:], OP.mult)
                        dst = probs_t.ap()[
                            h, pp * 256:(pp + 1) * 256, qsl
                        ].rearrange("(g r) q -> r g q", g=2)
                        nc.sync.dma_start(dst, e_t[:])

        epool.release()
        ph2.release()
        qkvp.release()

        ph3 = tc.alloc_tile_pool(name="ph3", bufs=1)
        ph3tmp = tc.alloc_tile_pool(name="ph3tmp", bufs=2)

        wotp_sb = ph3.tile([P, 8, H], f32r)
        for it in range(8):
            tmp = ph3tmp.tile([P, H], f32, tag="relu", name="tmp")
            nc.scalar.activation(tmp[:], wot_sb[:, it], AF.Relu)
            nc.vector.scalar_tensor_tensor(
                wotp_sb[:, it], tmp[:], g128[:, 0:1], wot_sb[:, it],
                OP.mult, OP.add)

        xn_r = xn.ap().rearrange("(t p) o -> p t o", p=P)
        out_r = out.ap().rearrange("(t p) o -> t p o", p=P)
        for st in range(8):
            xn_t = ph3tmp.tile([P, H], f32, tag="xn", name="xn_t", bufs=3)
            nc.sync.dma_start(xn_t[:], xn_r[:, st])
            nc.vector.tensor_tensor(xn_t[:], xn_t[:], bo_b[:], OP.add)
            pso = [ctxps.tile([P, 512], f32, tag="pj", name="pso")
                   for _ in range(2)]
            for it in range(8):
                lhsT = ctxt_sb[:, it, st * P:(st + 1) * P]
                for oc in range(2):
                    nc.tensor.matmul(
                        pso[oc][:], lhsT,
                        wotp_sb[:, it, oc * 512:(oc + 1) * 512],
                        start=(it == 0), stop=(it == 7))
            h_sb = ph3tmp.tile([P, H], f32, tag="h", name="h_sb")
            for oc in range(2):
                nc.vector.tensor_tensor(
                    h_sb[:, oc * 512:(oc + 1) * 512], pso[oc][:],
                    xn_t[:, oc * 512:(oc + 1) * 512], OP.add)
            stats = ph3tmp.tile([P, 2, 6], f32, tag="stats", name="stats")
            for g2 in range(2):
                nc.vector.bn_stats(stats[:, g2], h_sb[:, g2 * 512:(g2 + 1) * 512])
            mv = ph3tmp.tile([P, 2], f32, tag="mv", name="mv")
            nc.vector.bn_aggr(mv[:], stats[:])
            lnv = ph3tmp.tile([P, 1], f32, tag="lnv", name="lnv")
            nc.scalar.activation(lnv[:], mv[:, 1:2], AF.Ln, bias=eps_sb[:, 0:1])
            y = ph3tmp.tile([P, 1], f32, tag="y", name="y")
            nc.scalar.activation(y[:], lnv[:], AF.Exp, scale=-0.5)
            t1 = ph3tmp.tile([P, 1], f32, tag="t1", name="t1")
            nc.vector.tensor_tensor(t1[:], y[:], y[:], OP.mult)
            nc.vector.tensor_tensor(t1[:], t1[:], mv[:, 1:2], OP.mult)
            nc.vector.tensor_scalar(t1[:], t1[:], -0.5, 1.5, OP.mult, OP.add)
            nc.vector.tensor_tensor(y[:], y[:], t1[:], OP.mult)
            negm = ph3tmp.tile([P, 1], f32, tag="negm", name="negm")
            nc.vector.tensor_scalar_mul(negm[:], mv[:, 0:1], -1.0)
            o_sb = ph3tmp.tile([P, H], f32, tag="o_sb", name="o_sb")
            nc.vector.tensor_scalar(
                o_sb[:], h_sb[:], negm[:, 0:1], y[:, 0:1], OP.add, OP.mult)
            nc.sync.dma_start(out_r[st], o_sb[:])

        ph3tmp.release()
        ph3.release()
        wotpool.release()
        ctxps.release()
        stps.release()
        persist.release()
        constp.release()

    nc.compile()
    return nc


def _get_nc(probs_bf16=True):
    key = ("nc", probs_bf16)
    if key not in _CACHE:
        _CACHE[key] = _build(probs_bf16)
    return _CACHE[key]


def run(inputs, trace=False, probs_bf16=True):
    from concourse.bass_utils import run_bass_kernel_spmd

    nc = _get_nc(probs_bf16)
    X = np.ascontiguousarray(np.asarray(inputs["hidden_states"], np.float32))
    wqt = np.ascontiguousarray(np.asarray(inputs["Wq"], np.float32).T)
    wkt = np.ascontiguousarray(np.asarray(inputs["Wk"], np.float32).T)
    wvt = np.ascontiguousarray(np.asarray(inputs["Wv"], np.float32).T)
    wot = np.ascontiguousarray(np.asarray(inputs["Wo"], np.float32).T)
    bq = np.ascontiguousarray(np.asarray(inputs["bq"], np.float32))
    bk = np.ascontiguousarray(np.asarray(inputs["bk"], np.float32))
    bv = np.ascontiguousarray(np.asarray(inputs["bv"], np.float32))
    bo = np.ascontiguousarray(np.asarray(inputs["bo"], np.float32))
    gl = np.array([[np.float32(inputs["gamma_LN"])]], np.float32)

    in_maps = []
    for b in range(B):
        in_maps.append({
            "xt": np.ascontiguousarray(X[b].T),
            "xn": X[b],
            "wqt": wqt, "wkt": wkt, "wvt": wvt, "wot": wot,
            "bq": bq, "bk": bk, "bv": bv, "bo": bo,
            "g": gl,
        })
    res = run_bass_kernel_spmd(nc, in_maps, core_ids=list(range(B)),
                               trace=trace)
    out = np.stack([res.results[b]["out"] for b in range(B)])
    probs = np.stack([
        np.asarray(res.results[b]["probs_t"]).astype(np.float32).transpose(0, 2, 1)
        for b in range(B)
    ])
    return (out, probs), res


def kernel(**inputs):
    (out, probs), _ = run(inputs, trace=False)
    return out, probs


# revision 26
# speedup vs baseline: 1.4558x; 1.0040x over previous
"""Trainium2 Bass kernel for nn_AttentionBlock_88983132438589.

Math (value-level, all stop_gradient tricks removed):
  q = x @ Wq.T + bq ; k = x @ Wk.T + bk ; v = x @ Wv.T + bv
  scores = q @ k.T per head (NO 1/sqrt(d) scaling)
  probs  = softmax(scores)           (gamma_AH cancels in value)
  ctx    = probs @ v
  h      = ctx @ (Wo + gLN*relu(Wo)).T + (bo + gLN*relu(bo)) + x
  out    = (h - mean(h)) / sqrt(var(h) + 1e-12)
  returns (out, probs)

Sharding: data-parallel over batch, 1 batch element per NeuronCore (8 cores).

On-device layout is "transposed": Q^T,K^T [o, s] so the scores matmul
contracts over head_dim on partitions, softmax denominators come from an
ones-column appended to V in the ctx matmul (they land in ctx row 64), and
ctx^T directly feeds the output projection as the stationary operand.
probs are written to HBM as [h, k, q] (transposed); host transposes back.
"""

import sys
import numpy as np

sys.path.insert(0, "/opt/trn_rl_repo")

B, S, H = 8, 1024, 1024
NH, HD = 16, 64
P = 128
EPS = 1e-12

_CACHE = {}


def _build(probs_bf16=True):
    import concourse.bass as bass  # noqa
    import concourse.mybir as mybir
    import concourse.tile as tile
    from concourse import bacc

    f32 = mybir.dt.float32
    f32r = mybir.dt.float32r
    bf16 = mybir.dt.bfloat16
    AF = mybir.ActivationFunctionType
    OP = mybir.AluOpType
    pdt = bf16 if probs_bf16 else f32

    nc = bacc.Bacc("TRN2", target_bir_lowering=False, debug=False, num_devices=8)

    xt = nc.dram_tensor("xt", [H, S], f32, kind="ExternalInput")
    xn = nc.dram_tensor("xn", [S, H], f32, kind="ExternalInput")
    wqt = nc.dram_tensor("wqt", [H, H], f32, kind="ExternalInput")
    wkt = nc.dram_tensor("wkt", [H, H], f32, kind="ExternalInput")
    wvt = nc.dram_tensor("wvt", [H, H], f32, kind="ExternalInput")
    wot = nc.dram_tensor("wot", [H, H], f32, kind="ExternalInput")
    bq = nc.dram_tensor("bq", [H], f32, kind="ExternalInput")
    bk = nc.dram_tensor("bk", [H], f32, kind="ExternalInput")
    bv = nc.dram_tensor("bv", [H], f32, kind="ExternalInput")
    bo = nc.dram_tensor("bo", [H], f32, kind="ExternalInput")
    g = nc.dram_tensor("g", [1, 1], f32, kind="ExternalInput")

    out = nc.dram_tensor("out", [S, H], f32, kind="ExternalOutput")
    probs_t = nc.dram_tensor("probs_t", [NH, S, S], pdt, kind="ExternalOutput")

    with tile.TileContext(nc) as tc:
        constp = tc.alloc_tile_pool(name="constp", bufs=1)
        rows = tc.alloc_tile_pool(name="rows", bufs=1)

        bq_sb = constp.tile([P, 8], f32)
        nc.sync.dma_start(bq_sb[:], bq.ap().rearrange("(t p) -> p t", p=P))
        bk_sb = constp.tile([P, 8], f32)
        nc.sync.dma_start(bk_sb[:], bk.ap().rearrange("(t p) -> p t", p=P))
        g_sb = constp.tile([1, 1], f32)
        nc.sync.dma_start(g_sb[:], g.ap())
        g128 = constp.tile([P, 1], f32)
        nc.gpsimd.partition_broadcast(g128[:], g_sb[:])
        eps_sb = constp.tile([P, 1], f32)
        nc.vector.memset(eps_sb[:], EPS)

        bv_row = rows.tile([1, H], f32)
        nc.sync.dma_start(bv_row[:], bv.ap()[None, :])
        bv_b = constp.tile([P, H], f32)
        nc.gpsimd.partition_broadcast(bv_b[:], bv_row[:])
        bo_row = rows.tile([1, H], f32)
        nc.sync.dma_start(bo_row[:], bo.ap()[None, :])
        bo_relu = rows.tile([1, H], f32)
        nc.scalar.activation(bo_relu[:], bo_row[:], AF.Relu)
        bo_p_row = rows.tile([1, H], f32)
        nc.vector.scalar_tensor_tensor(
            bo_p_row[:], bo_relu[:], g_sb[:], bo_row[:], OP.mult, OP.add)
        bo_b = constp.tile([P, H], f32)
        nc.gpsimd.partition_broadcast(bo_b[:], bo_p_row[:])
        rows.release()

        qkvp = tc.alloc_tile_pool(name="qkvp", bufs=1)
        qt_sb = qkvp.tile([P, 8, S], f32r)
        kt_sb = qkvp.tile([P, 8, S], f32r)
        v_sb = qkvp.tile([P, 8, NH * 65], bf16)

        stps = tc.alloc_tile_pool(name="stps", bufs=2, space="PSUM")
        ctxps = tc.alloc_tile_pool(name="ctxps", bufs=4, space="PSUM")

        ph1 = tc.alloc_tile_pool(name="ph1", bufs=1)
        wpool = tc.alloc_tile_pool(name="wpool", bufs=2)

        xt_sb = ph1.tile([P, 8, S], f32r)
        xt_r = xt.ap().rearrange("(t p) s -> p t s", p=P)
        wv_sb = wpool.tile([P, 8, H], f32r, tag="w", name="wv_sb")
        wv_r = wvt.ap().rearrange("(t p) o -> p t o", p=P)
        for it in range(8):
            nc.gpsimd.dma_start(xt_sb[:, it], xt_r[:, it])
            nc.gpsimd.dma_start(wv_sb[:, it], wv_r[:, it])
        for st in range(8):
            ones_ap = v_sb[:, st].rearrange("p (h d) -> p h d", d=65)[:, :, 64:65]
            nc.vector.memset(ones_ap, 1.0)

        for st in range(8):
            for oc in range(2):
                psv = ctxps.tile([P, 512], f32, tag="pj", name="psv")
                for it in range(8):
                    nc.tensor.matmul(
                        psv[:], xt_sb[:, it, st * P:(st + 1) * P],
                        wv_sb[:, it, oc * 512:(oc + 1) * 512],
                        start=(it == 0), stop=(it == 7))
                v_out = v_sb[:, st].rearrange(
                    "p (h d) -> p h d", d=65)[:, oc * 8:(oc + 1) * 8, 0:64]
                nc.vector.tensor_tensor(
                    v_out,
                    psv[:].rearrange("p (h d) -> p h d", d=64),
                    bv_b[:, oc * 512:(oc + 1) * 512].rearrange(
                        "p (h d) -> p h d", d=64),
                    OP.add)

        wq_sb = wpool.tile([P, 8, H], f32r, tag="w", name="wq_sb")
        wq_r = wqt.ap().rearrange("(t p) o -> p t o", p=P)
        wk_sb = wpool.tile([P, 8, H], f32r, tag="w", name="wk_sb")
        wk_r = wkt.ap().rearrange("(t p) o -> p t o", p=P)
        for it in range(8):
            nc.gpsimd.dma_start(wq_sb[:, it], wq_r[:, it])
            nc.gpsimd.dma_start(wk_sb[:, it], wk_r[:, it])
        for ot in range(8):
            for w_sb, slab, bias in ((wq_sb, qt_sb, bq_sb), (wk_sb, kt_sb, bk_sb)):
                pst = [ctxps.tile([P, 512], f32, tag="pj", name="pst")
                       for _ in range(2)]
                for it in range(8):
                    lhsT = w_sb[:, it, ot * P:(ot + 1) * P]
                    for sc in range(2):
                        nc.tensor.matmul(
                            pst[sc][:], lhsT,
                            xt_sb[:, it, sc * 512:(sc + 1) * 512],
                            start=(it == 0), stop=(it == 7))
                for sc in range(2):
                    nc.vector.tensor_scalar_add(
                        slab[:, ot, sc * 512:(sc + 1) * 512],
                        pst[sc][:], bias[:, ot:ot + 1])

        wpool.release()
        ph1.release()

        persist = tc.alloc_tile_pool(name="persist", bufs=1, side="right")
        ctxt_sb = persist.tile([P, 8, H], f32r)

        wotpool = tc.alloc_tile_pool(name="wotpool", bufs=1, side="right")
        wot_sb = wotpool.tile([P, 8, H], f32)
        wot_r = wot.ap().rearrange("(t p) o -> p t o", p=P)
        for it in range(8):
            nc.sync.dma_start(wot_sb[:, it], wot_r[:, it])

        ph2 = tc.alloc_tile_pool(name="ph2", bufs=3)
        epool = tc.alloc_tile_pool(name="epool", bufs=18)

        for qc in range(2):
            qsl = slice(qc * 512, (qc + 1) * 512)
            for hp in range(8):
                heads = [(2 * hp, 0), (2 * hp + 1, 64)]
                ctx_ps = [ctxps.tile([65, 512], f32, tag="pj", name="ctx_ps")
                          for _ in range(2)]
                e_tiles = [[None] * 4, [None] * 4]
                for pp in range(4):
                    st_ps = [stps.tile([P, 1024], f32, tag="st", name="st_ps")
                             for _ in range(2)]
                    for gg in range(2):
                        kt = 2 * pp + gg
                        ksl = slice(kt * P, (kt + 1) * P)
                        for hi, (h, po) in enumerate(heads):
                            nc.tensor.matmul(
                                st_ps[hi][:, gg * 512:(gg + 1) * 512],
                                kt_sb[po:po + 64, hp, ksl],
                                qt_sb[po:po + 64, hp, qsl],
                                start=True, stop=True)
                    for hi, (h, po) in enumerate(heads):
                        e_t = epool.tile([P, 2, 512], bf16, tag="E", name="e_t")
                        nc.scalar.activation(
                            e_t[:],
                            st_ps[hi][:].rearrange("p (g q) -> p g q", g=2),
                            AF.Exp)
                        e_tiles[hi][pp] = e_t
                    for gg in range(2):
                        kt = 2 * pp + gg
                        for hi, (h, po) in enumerate(heads):
                            nc.tensor.matmul(
                                ctx_ps[hi][:],
                                v_sb[:, kt, h * 65:(h + 1) * 65],
                                e_tiles[hi][pp][:, gg],
                                start=(kt == 0), stop=(kt == 7))
                # batch Ln then Exp across the head pair: 2 ACT table
                # switches per hp-iteration instead of 4
                # 1/sum on DVE (reciprocal_approx_fast, ~51ULP ample for
                # the bf16 probs path); ACT Copy is set-neutral so no table
                # reloads interleave with the big softmax Exps
                rrows = []
                for hi in range(2):
                    srow = ph2.tile([1, 512], f32, tag="srow", name="srow")
                    nc.scalar.activation(srow[:], ctx_ps[hi][64:65, :],
                                         AF.Copy)
                    rrow = ph2.tile([1, 512], f32, tag="rrow", name="rrow")
                    nc.vector.reciprocal_approx_fast(rrow[:], srow[:])
                    rbf = ph2.tile([1, 512], bf16, tag="rbf", name="rbf")
                    nc.vector.tensor_copy(rbf[:], rrow[:])
                    rrows.append(rbf)
                for hi, (h, po) in enumerate(heads):
                    rb = ph2.tile([P, 512], bf16, tag="rb", name="rb")
                    nc.gpsimd.partition_broadcast(rb[:], rrows[hi][:])
                    nc.vector.tensor_tensor(
                        ctxt_sb[po:po + 64, hp, qsl],
                        ctx_ps[hi][0:64, :], rb[0:64, :], OP.mult)
                    for pp in range(4):
                        e_t = e_tiles[hi][pp]
                        for gg in range(2):
                            eng = nc.gpsimd if pp == 3 else nc.vector
                            eng.tensor_tensor(
                                e_t[:, gg], e_t[:, gg], rb[:], OP.mult)
                        dst = probs_t.ap()[
                            h, pp * 256:(pp + 1) * 256, qsl
                        ].rearrange("(g r) q -> r g q", g=2)
                        nc.sync.dma_start(dst, e_t[:])

        epool.release()
        ph2.release()
        qkvp.release()

        ph3 = tc.alloc_tile_pool(name="ph3", bufs=1)
        ph3tmp = tc.alloc_tile_pool(name="ph3tmp", bufs=2)

        wotp_sb = ph3.tile([P, 8, H], f32r)
        for it in range(8):
            tmp = ph3tmp.tile([P, H], f32, tag="relu", name="tmp")
            nc.scalar.activation(tmp[:], wot_sb[:, it], AF.Relu)
            nc.vector.scalar_tensor_tensor(
                wotp_sb[:, it], tmp[:], g128[:, 0:1], wot_sb[:, it],
                OP.mult, OP.add)

        xn_r = xn.ap().rearrange("(t p) o -> p t o", p=P)
        out_r = out.ap().rearrange("(t p) o -> t p o", p=P)
        for st in range(8):
            xn_t = ph3tmp.tile([P, H], f32, tag="xn", name="xn_t", bufs=3)
            nc.sync.dma_start(xn_t[:], xn_r[:, st])
            nc.vector.tensor_tensor(xn_t[:], xn_t[:], bo_b[:], OP.add)
            pso = [ctxps.tile([P, 512], f32, tag="pj", name="pso")
                   for _ in range(2)]
            for it in range(8):
                lhsT = ctxt_sb[:, it, st * P:(st + 1) * P]
                for oc in range(2):
                    nc.tensor.matmul(
                        pso[oc][:], lhsT,
                        wotp_sb[:, it, oc * 512:(oc + 1) * 512],
                        start=(it == 0), stop=(it == 7))
            h_sb = ph3tmp.tile([P, H], f32, tag="h", name="h_sb")
            for oc in range(2):
                nc.vector.tensor_tensor(
                    h_sb[:, oc * 512:(oc + 1) * 512], pso[oc][:],
                    xn_t[:, oc * 512:(oc + 1) * 512], OP.add)
            stats = ph3tmp.tile([P, 2, 6], f32, tag="stats", name="stats")
            for g2 in range(2):
                nc.vector.bn_stats(stats[:, g2], h_sb[:, g2 * 512:(g2 + 1) * 512])
            mv = ph3tmp.tile([P, 2], f32, tag="mv", name="mv")
            nc.vector.bn_aggr(mv[:], stats[:])
            lnv = ph3tmp.tile([P, 1], f32, tag="lnv", name="lnv")
            nc.scalar.activation(lnv[:], mv[:, 1:2], AF.Ln, bias=eps_sb[:, 0:1])
            y = ph3tmp.tile([P, 1], f32, tag="y", name="y")
            nc.scalar.activation(y[:], lnv[:], AF.Exp, scale=-0.5)
            t1 = ph3tmp.tile([P, 1], f32, tag="t1", name="t1")
            nc.vector.tensor_tensor(t1[:], y[:], y[:], OP.mult)
            nc.vector.tensor_tensor(t1[:], t1[:], mv[:, 1:2], OP.mult)
            nc.vector.tensor_scalar(t1[:], t1[:], -0.5, 1.5, OP.mult, OP.add)
            nc.vector.tensor_tensor(y[:], y[:], t1[:], OP.mult)
            negm = ph3tmp.tile([P, 1], f32, tag="negm", name="negm")
            nc.vector.tensor_scalar_mul(negm[:], mv[:, 0:1], -1.0)
            o_sb = ph3tmp.tile([P, H], f32, tag="o_sb", name="o_sb")
            nc.vector.tensor_scalar(
                o_sb[:], h_sb[:], negm[:, 0:1], y[:, 0:1], OP.add, OP.mult)
            nc.sync.dma_start(out_r[st], o_sb[:])

        ph3tmp.release()
        ph3.release()
        wotpool.release()
        ctxps.release()
        stps.release()
        persist.release()
        constp.release()

    nc.compile()
    return nc


def _get_nc(probs_bf16=True):
    key = ("nc", probs_bf16)
    if key not in _CACHE:
        _CACHE[key] = _build(probs_bf16)
    return _CACHE[key]


def run(inputs, trace=False, probs_bf16=True):
    from concourse.bass_utils import run_bass_kernel_spmd

    nc = _get_nc(probs_bf16)
    X = np.ascontiguousarray(np.asarray(inputs["hidden_states"], np.float32))
    wqt = np.ascontiguousarray(np.asarray(inputs["Wq"], np.float32).T)
    wkt = np.ascontiguousarray(np.asarray(inputs["Wk"], np.float32).T)
    wvt = np.ascontiguousarray(np.asarray(inputs["Wv"], np.float32).T)
    wot = np.ascontiguousarray(np.asarray(inputs["Wo"], np.float32).T)
    bq = np.ascontiguousarray(np.asarray(inputs["bq"], np.float32))
    bk = np.ascontiguousarray(np.asarray(inputs["bk"], np.float32))
    bv = np.ascontiguousarray(np.asarray(inputs["bv"], np.float32))
    bo = np.ascontiguousarray(np.asarray(inputs["bo"], np.float32))
    gl = np.array([[np.float32(inputs["gamma_LN"])]], np.float32)

    in_maps = []
    for b in range(B):
        in_maps.append({
            "xt": np.ascontiguousarray(X[b].T),
            "xn": X[b],
            "wqt": wqt, "wkt": wkt, "wvt": wvt, "wot": wot,
            "bq": bq, "bk": bk, "bv": bv, "bo": bo,
            "g": gl,
        })
    res = run_bass_kernel_spmd(nc, in_maps, core_ids=list(range(B)),
                               trace=trace)
    out = np.stack([res.results[b]["out"] for b in range(B)])
    probs = np.stack([
        np.asarray(res.results[b]["probs_t"]).astype(np.float32).transpose(0, 2, 1)
        for b in range(B)
    ])
    return (out, probs), res


def kernel(**inputs):
    (out, probs), _ = run(inputs, trace=False)
    return out, probs
